# revision 1
# baseline (speedup 1.0000x reference)
"""Maxish pooling kernel for Trainium2 (8 NeuronCores, data-parallel).

Reference math (per row of length N):
    m  = max(x)
    rt = (x - m) / (m + 1e-8)
    pos = m * sum(exp((1+s)*rt)) / sum(exp(s*rt))   # softmax identity
    neg = m                                          # softmax sums to 1
    out = m > 0 ? pos : (m < 0 ? m : 0)

Layout: rows on partitions (128/tile), N=256 on the free axis.
Fast path (s == 1): one ACT exp pass with per-partition scale/bias and
fused accum (-> sum u), one DVE tensor_tensor_reduce (-> sum u^2, bf16).
"""

import numpy as np

P = 128
N = 256
SMALL = 1e-8


def _build(n_rows: int, s: float, G: int = 16, dt_u=None, x_bufs: int = 3,
           u_bufs: int = 2, act_tiles: int = 2, smalls_gpsimd: bool = False):
    from concourse import bacc, mybir
    from concourse import masks
    from concourse.tile import TileContext

    f32 = mybir.dt.float32
    if dt_u is None:
        dt_u = mybir.dt.float32
    Act = mybir.ActivationFunctionType
    Alu = mybir.AluOpType
    Ax = mybir.AxisListType

    assert n_rows % (P * G) == 0
    T = n_rows // P          # tiles of [128, N]
    C = T // G               # chunks of G tiles
    fast = (s == 1.0)

    nc = bacc.Bacc("TRN2", target_bir_lowering=False, debug=False,
                   num_devices=8)
    x_d = nc.declare_dram_parameter("x", [n_rows, N], f32, isOutput=False)
    out_d = nc.declare_dram_parameter("out", [n_rows], f32, isOutput=True)

    with TileContext(nc) as tc:
        with (
            tc.tile_pool(name="xp", bufs=x_bufs) as xp,
            tc.tile_pool(name="up", bufs=u_bufs) as up,
            tc.tile_pool(name="stat", bufs=1) as statp,
            tc.tile_pool(name="consts", bufs=4) as cpool,
            tc.tile_pool(name="psum", bufs=2, space="PSUM") as psp,
        ):
            M = statp.tile([P, T], f32, tag="M")       # per-row max
            S1 = statp.tile([P, T], f32, tag="S1")     # sum exp((1+s)rt)
            S2 = statp.tile([P, T], f32, tag="S2")     # sum exp(s rt)
            R = statp.tile([P, T], f32, tag="R")       # final per-row result
            RT = statp.tile([P, T], f32, tag="RT")     # transposed result
            MK = statp.tile([P, T], mybir.dt.uint8, tag="MK")  # m>0 mask

            ident = statp.tile([P, P], f32, tag="ident")
            masks.make_identity(nc, ident[:])

            for c in range(C):
                xt = xp.tile([P, G * N], f32, tag="x")
                src = x_d[c * G * P:(c + 1) * G * P, :].rearrange(
                    "(g p) n -> p g n", p=P)
                nc.sync.dma_start(
                    out=xt[:].rearrange("p (g n) -> p g n", n=N), in_=src)

                x3 = xt[:].rearrange("p (g n) -> p g n", n=N)
                mg = M[:, c * G:(c + 1) * G]
                nc.vector.tensor_reduce(out=mg, in_=x3, axis=Ax.X,
                                        op=Alu.max)
                # per-chunk consts in a versioned pool tile so ACT's reads
                # of chunk c don't serialize against DVE writing chunk c+1
                cb = cpool.tile([P, 7 * G], f32, tag="cb")
                rg = cb[:, 0:G]
                bg = cb[:, G:2 * G]
                # rg = 1 / (m + eps), clamped to >= 0 so the exponent
                # r*(x-m) stays <= 0 (m<0 rows are masked later; without
                # the clamp they can overflow exp)
                nc.vector.tensor_scalar_add(rg, mg, SMALL)
                nc.vector.reciprocal(rg, rg)
                nc.vector.tensor_scalar_max(rg, rg, 0.0)
                # bg = (m * -1) * rg = -m/(m+eps)
                sm = nc.gpsimd if smalls_gpsimd else nc.vector
                nc.vector.scalar_tensor_tensor(
                    out=bg, in0=mg, scalar=-1.0, in1=rg,
                    op0=Alu.mult, op1=Alu.mult)

                if fast:
                    # last `b` tiles of each chunk are ACT-only (two
                    # exp+accum passes); the rest use bn_stats on DVE
                    b = min(act_tiles, G)
                    ga = G - b
                    ut = up.tile([P, G * N], dt_u, tag="u")
                    for g in range(ga):
                        fs = slice(g * N, (g + 1) * N)
                        nc.scalar.activation(
                            out=ut[:, fs], in_=xt[:, fs], func=Act.Exp,
                            scale=rg[:, g:g + 1], bias=bg[:, g:g + 1])
                    for g in range(ga, G):
                        fs = slice(g * N, (g + 1) * N)
                        j = c * G + g
                        nc.scalar.activation(
                            out=ut[:, fs], in_=xt[:, fs], func=Act.Exp,
                            scale=rg[:, g:g + 1], bias=bg[:, g:g + 1],
                            accum_out=S2[:, j:j + 1])
                        # sum exp(2rt) == sum u^2 via Square (no extra
                        # per-partition consts needed)
                        nc.scalar.activation(
                            out=ut[:, fs], in_=ut[:, fs], func=Act.Square,
                            accum_out=S1[:, j:j + 1])
                    # both sums via per-tile bn_stats over u:
                    # S2 = n*mean, S1 = n*var + mean*S2
                    bst = cpool.tile([P, G * 6], f32, tag="bst")
                    for g in range(ga):
                        nc.vector.bn_stats(
                            out=bst[:, g * 6:(g + 1) * 6],
                            in_=ut[:, g * N:(g + 1) * N])
                    # per-tile 6-tuple: [n_e, mu_e, M2_e, n_o, mu_o, M2_o]
                    # S2 = 128*(mu_e+mu_o); S1 = M2_e+M2_o+128*(mu_e^2+mu_o^2)
                    bsg = bst[:, :ga * 6].rearrange("p (g s) -> p s g", s=6)
                    mu_e, m2_e = bsg[:, 1], bsg[:, 2]
                    mu_o, m2_o = bsg[:, 4], bsg[:, 5]
                    s2c = S2[:, c * G:c * G + ga]
                    s1c = S1[:, c * G:c * G + ga]
                    t1 = cb[:, 2 * G:2 * G + ga]
                    t2 = cb[:, 3 * G:3 * G + ga]
                    t3 = cb[:, 4 * G:4 * G + ga]
                    half = float(N // 2)
                    sm.tensor_tensor(t1, mu_e, mu_o, op=Alu.add)
                    nc.vector.tensor_scalar_mul(s2c, t1, half)
                    sm.tensor_tensor(t2, mu_e, mu_e, op=Alu.mult)
                    sm.tensor_tensor(t3, mu_o, mu_o, op=Alu.mult)
                    sm.tensor_tensor(t2, t2, t3, op=Alu.add)
                    sm.tensor_tensor(t1, m2_e, m2_o, op=Alu.add)
                    nc.vector.scalar_tensor_tensor(
                        out=s1c, in0=t2, scalar=half, in1=t1,
                        op0=Alu.mult, op1=Alu.add)
                else:
                    c1 = cb[:, 2 * G:3 * G]
                    b1 = cb[:, 3 * G:4 * G]
                    nc.vector.tensor_scalar_mul(c1, rg, 1.0 + s)
                    nc.vector.tensor_scalar_mul(b1, bg, 1.0 + s)
                    nc.vector.tensor_scalar_mul(rg, rg, s)
                    nc.vector.tensor_scalar_mul(bg, bg, s)
                    ut = up.tile([P, G * N], dt_u, tag="u")
                    for g in range(G):
                        fs = slice(g * N, (g + 1) * N)
                        j = c * G + g
                        nc.scalar.activation(
                            out=ut[:, fs], in_=xt[:, fs], func=Act.Exp,
                            scale=rg[:, g:g + 1], bias=bg[:, g:g + 1],
                            accum_out=S2[:, j:j + 1])
                        nc.scalar.activation(
                            out=ut[:, fs], in_=xt[:, fs], func=Act.Exp,
                            scale=c1[:, g:g + 1], bias=b1[:, g:g + 1],
                            accum_out=S1[:, j:j + 1])

            # pos = m * S1 / S2 ; out = m > 0 ? pos : (m < 0 ? m : 0)
            nc.vector.reciprocal(S2[:], S2[:])
            nc.vector.tensor_tensor(S1[:], S1[:], S2[:], op=Alu.mult)
            nc.vector.tensor_tensor(S1[:], S1[:], M[:], op=Alu.mult)
            # mask of m > 0 (uint8 — CopyPredicated needs an int mask)
            nc.vector.tensor_scalar(MK[:], M[:], 0.0, None, op0=Alu.is_gt)
            nc.vector.tensor_copy(R[:], M[:])
            nc.vector.copy_predicated(out=R[:], mask=MK[:], data=S1[:])

            # transpose R [128, T] -> RT so the store DMA has >=512B runs:
            # out row = t*128 + p ; RT[t_lo, k*128 + p] with t = k*128 + t_lo
            assert T % P == 0
            KB = T // P
            for k in range(KB):
                pt = psp.tile([P, P], f32, tag="pt")
                nc.tensor.transpose(pt[:], R[:, k * P:(k + 1) * P], ident[:])
                nc.vector.tensor_copy(RT[:, k * P:(k + 1) * P], pt[:])
            nc.sync.dma_start(
                out=out_d[:].rearrange("(k t p) -> t k p", k=KB, p=P),
                in_=RT[:].rearrange("t (k p) -> t k p", p=P))

    nc.compile()
    return nc


def _run(x: np.ndarray, scale: np.ndarray, trace: bool = False,
         build_kw: dict | None = None, **kw):
    from concourse.bass_utils import run_bass_kernel_spmd

    n_cores = 8
    B, Tm, X, Nn = x.shape          # 32, 256, 64, 256
    assert Nn == N
    rows = B * Tm * X
    rows_per_core = rows // n_cores
    s = float(np.asarray(scale))

    nc = _build(rows_per_core, s, **(build_kw or {}))
    xs = np.ascontiguousarray(np.asarray(x, dtype=np.float32)).reshape(
        n_cores, rows_per_core, N)
    in_maps = [{"x": xs[i]} for i in range(n_cores)]
    res = run_bass_kernel_spmd(nc, in_maps, list(range(n_cores)),
                               trace=trace, **kw)
    out = np.concatenate([r["out"].reshape(-1) for r in res.results], axis=0)
    return out.reshape(B, Tm, X).astype(np.float32), res


def kernel(x: np.ndarray, scale: np.ndarray) -> np.ndarray:
    return _run(x, scale)[0]

